# revision 37
# baseline (speedup 1.0000x reference)
"""Trainium2 Bass kernel for CryptoAttentionLayer.

Computation (per batch element b, per token t):
    Q = x @ Wq + bq ; K = x @ Wk + bk ; V = x @ Wv + bv    (4 heads x 256)
    S[h,g]   = Q_h . K_g / 16                               (per-token 4x4 scores)
    W        = softmax_g(S)
    att_h    = sum_g W[h,g] * V_g
    out      = att @ Wo + bo

Sharding: data-parallel over B=8 across 8 NeuronCores; weights replicated.

Design notes:
  - Q/K projections run in fp8e4 with perf_mode=DoubleRow (2 fp8/cell, K=256
    per pass).  x is pre-scaled by 2^4 and Wq/Wk by 2^7 so values sit in the
    e4m3 normal range; the 2^-22 descale plus the 1/sqrt(256) score scale are
    applied on the score path.  Q/K biases are folded EXACTLY into a tiny
    N=16 "S_bias" matmul: S = Q0.K0/16 + x @ Msb + c0 with
    Msb = (Wq@Mk + Wk@Mq)/16, c0[h,g] = bq_h.bk_g/16.
  - V/O projections run in bf16.  V bias rides a ones-row matmul into the
    same PSUM accumulation; O bias is added by the DVE during eviction.
  - Scores: one DVE tensor_tensor (Q broadcast over g x K broadcast over h)
    + one tensor_reduce.  Softmax over 4 head pairs, no max-subtract
    (scores are O(1)).
  - Head mixing on DVE in bf16; PE transposes att for the O projection.
  - 2-deep software pipeline: transposes + O-projection of tile i-2 are
    issued after the projections of tile i, so the PE never waits on the
    vector engine's attention math.
"""

import math

import numpy as np
import ml_dtypes

import concourse.bass as bass
import concourse.tile as tile
import concourse.mybir as mybir
from concourse import bacc
from concourse.bass_utils import run_bass_kernel_spmd
from concourse.masks import make_identity

B, N, D = 8, 4096, 1024
NUM_HEADS, HEAD_DIM = 4, 256
P = 128
NT = N // P          # 32 token tiles per core
KC = D // P          # 8 contraction chunks of 128
KC2 = KC // 2        # 4 double-chunks for fp8 DoubleRow
F32 = mybir.dt.float32
BF16 = mybir.dt.bfloat16
FP8 = mybir.dt.float8e4
ALU = mybir.AluOpType
ACTF = mybir.ActivationFunctionType
DR = mybir.MatmulPerfMode.DoubleRow

X_SCALE = 16.0       # 2^4  : x pre-scale for fp8
W_SCALE = 128.0      # 2^7  : Wq/Wk pre-scale for fp8
DESCALE = 1.0 / (X_SCALE * X_SCALE * W_SCALE * W_SCALE * math.sqrt(HEAD_DIM))

_CACHED_NC = None

# test.py can set these to capture a perfetto trace + HW exec time; the
# grading harness never touches them.
TRACE = False
TRACE_DIR = None
LAST_RESULT = None


def build_nc():
    nc = bacc.Bacc(None, target_bir_lowering=False)

    xt8_d = nc.dram_tensor("xt8", [NT, P, KC2, 2, P], FP8, kind="ExternalInput")
    xtbf_d = nc.dram_tensor("xtbf", [NT, P, KC, P], BF16, kind="ExternalInput")
    wq8_d = nc.dram_tensor("wq8", [P, KC2, 2, D], FP8, kind="ExternalInput")
    wk8_d = nc.dram_tensor("wk8", [P, KC2, 2, D], FP8, kind="ExternalInput")
    wv_d = nc.dram_tensor("wv", [P, KC, D], BF16, kind="ExternalInput")
    wo_d = nc.dram_tensor("wo", [P, KC, D], BF16, kind="ExternalInput")
    # replicated bias rows: q (x2^11), k (x2^11), v, o
    brep_d = nc.dram_tensor("brep", [P, 4, D], BF16, kind="ExternalInput")
    out_d = nc.dram_tensor("out", [N, D], F32, kind="ExternalOutput")

    with tile.TileContext(nc) as tc:
        with (
            tc.tile_pool(name="consts", bufs=1) as consts,
            tc.tile_pool(name="xt8", bufs=2) as xt8_pool,
            tc.tile_pool(name="xtbf", bufs=2) as xtbf_pool,
            tc.tile_pool(name="qk", bufs=2) as qk_pool,
            tc.tile_pool(name="v", bufs=2) as v_pool,
            tc.tile_pool(name="att", bufs=3) as att_pool,
            tc.tile_pool(name="attT", bufs=2) as attT_pool,
            tc.tile_pool(name="o", bufs=2) as o_pool,
            tc.tile_pool(name="small", bufs=2) as small,
            tc.tile_pool(name="psum", bufs=2, space="PSUM") as psum,
        ):
            wq8_sb = consts.tile([P, KC2, 2, D], FP8)
            wk8_sb = consts.tile([P, KC2, 2, D], FP8)
            wv_sb = consts.tile([P, KC, D], BF16)
            wo_sb = consts.tile([P, KC, D], BF16)
            brep_sb = consts.tile([P, 4, D], BF16)
            # critical-path weights on the sync DMA queue (ahead of the first
            # x tiles); the rest on the scalar hwdge queue.
            nc.sync.dma_start(wq8_sb, wq8_d[:])
            nc.scalar.dma_start(wk8_sb, wk8_d[:])
            nc.scalar.dma_start(wv_sb, wv_d[:])
            nc.scalar.dma_start(brep_sb, brep_d[:])
            nc.scalar.dma_start(wo_sb, wo_d[:])

            ident = consts.tile([P, P], BF16)
            make_identity(nc, ident)

            atts = [None] * NT
            attTs = [None] * NT

            def proj_phase(t):
                xt8 = xt8_pool.tile([P, KC2, 2, P], FP8, tag="xt8")
                nc.sync.dma_start(xt8, xt8_d[t])
                xtbf = xtbf_pool.tile([P, KC, P], BF16, tag="xtbf")
                nc.sync.dma_start(xtbf, xtbf_d[t])

                # ---- Q/K projections: fp8 DoubleRow, K=256 per pass ----
                q_sb = qk_pool.tile([P, D], BF16, tag="q")
                k_sb = qk_pool.tile([P, D], BF16, tag="k")
                for qk, (w8, dst) in enumerate(
                        ((wq8_sb, q_sb), (wk8_sb, k_sb))):
                    ps0 = psum.tile([P, 512], F32, tag="qk", bufs=3)
                    ps1 = psum.tile([P, 512], F32, tag="qk", bufs=3)
                    for c in range(KC2):
                        nc.tensor.matmul(
                            ps0, xt8[:, c], w8[:, c, :, 0:512],
                            start=(c == 0), stop=(c == KC2 - 1), perf_mode=DR,
                        )
                        nc.tensor.matmul(
                            ps1, xt8[:, c], w8[:, c, :, 512:1024],
                            start=(c == 0), stop=(c == KC2 - 1), perf_mode=DR,
                        )
                    nc.scalar.copy(dst[:, 0:512], ps0)
                    nc.scalar.copy(dst[:, 512:1024], ps1)
                    # bias (pre-scaled by 2^11) on the otherwise-idle gpsimd
                    nc.gpsimd.tensor_tensor(
                        out=dst, in0=dst, in1=brep_sb[:, qk], op=ALU.add)

                # ---- V projection (bf16) + S_bias (N=16), shared ldweights ----
                v_ps0 = psum.tile([P, 512], F32, tag="b", bufs=2)
                v_ps1 = psum.tile([P, 512], F32, tag="b", bufs=2)
                for k in range(KC):
                    nc.tensor.matmul(v_ps0, xtbf[:, k], wv_sb[:, k, 0:512],
                                     start=(k == 0), stop=(k == KC - 1))
                    nc.tensor.matmul(v_ps1, xtbf[:, k], wv_sb[:, k, 512:1024],
                                     start=(k == 0), stop=(k == KC - 1))
                v_sb = v_pool.tile([P, D], BF16, tag="v")
                nc.scalar.copy(v_sb[:, 0:512], v_ps0)
                nc.scalar.copy(v_sb[:, 512:1024], v_ps1)
                nc.gpsimd.tensor_tensor(
                    out=v_sb, in0=v_sb, in1=brep_sb[:, 2], op=ALU.add)

                # ---- scores: prod = Q_h * K_g (bcast), tree-reduce over d ----
                # (tensor_reduce ADD runs at 1x; TT adds hit the 2x bf16 mode,
                # so fold 256 -> 128 -> 64 with TT before the final reduce)
                prod = small.tile([P, NUM_HEADS, NUM_HEADS, HEAD_DIM], BF16,
                                  tag="prod")
                q4 = q_sb.rearrange("p (h d) -> p h d", h=NUM_HEADS)
                k4 = k_sb.rearrange("p (g d) -> p g d", g=NUM_HEADS)
                nc.vector.tensor_tensor(
                    out=prod,
                    in0=q4[:, :, None, :].to_broadcast(
                        (P, NUM_HEADS, NUM_HEADS, HEAD_DIM)),
                    in1=k4[:, None, :, :].to_broadcast(
                        (P, NUM_HEADS, NUM_HEADS, HEAD_DIM)),
                    op=ALU.mult,
                )
                pr2 = prod.rearrange("p h g (i d) -> p (h g) i d", i=2)
                fold1 = small.tile([P, 16, 128], BF16, tag="fold1")
                nc.vector.tensor_tensor(
                    out=fold1, in0=pr2[:, :, 0], in1=pr2[:, :, 1], op=ALU.add)
                f2 = fold1.rearrange("p q (i d) -> p q i d", i=2)
                fold2 = small.tile([P, 16, 64], BF16, tag="fold2")
                nc.vector.tensor_tensor(
                    out=fold2, in0=f2[:, :, 0], in1=f2[:, :, 1], op=ALU.add)
                s_raw = small.tile([P, 16], F32, tag="sraw")
                nc.vector.tensor_reduce(
                    out=s_raw.rearrange("p (h g) -> p h g", g=NUM_HEADS),
                    in_=fold2.rearrange("p (h g) d -> p h g d", g=NUM_HEADS),
                    axis=mybir.AxisListType.X,
                    op=ALU.add,
                )
                # ---- softmax over g (scores are O(1); no max-subtract);
                # the fp8/score descale rides the activation's scale ----
                e_sb = small.tile([P, 16], F32, tag="e")
                nc.scalar.activation(e_sb, s_raw, ACTF.Exp, scale=DESCALE)
                sums = small.tile([P, NUM_HEADS], F32, tag="sums")
                nc.vector.tensor_reduce(
                    out=sums,
                    in_=e_sb.rearrange("p (h g) -> p h g", g=NUM_HEADS),
                    axis=mybir.AxisListType.X,
                    op=ALU.add,
                )
                rec = small.tile([P, NUM_HEADS], F32, tag="rec")
                nc.vector.reciprocal(rec, sums)
                w_sm = small.tile([P, 16], F32, tag="w")
                nc.vector.tensor_tensor(
                    out=w_sm.rearrange("p (h g) -> p h g", g=NUM_HEADS),
                    in0=e_sb.rearrange("p (h g) -> p h g", g=NUM_HEADS),
                    in1=rec[:, :, None].to_broadcast(
                        (P, NUM_HEADS, NUM_HEADS)),
                    op=ALU.mult,
                )

                # ---- head mixing: att_h = sum_g w[h,g] * V_g (bf16) ----
                # g-outer so consecutive DVE ops are independent (no
                # serialization on the per-head accumulation chain); head 3
                # runs on the otherwise-idle gpsimd engine.
                att = att_pool.tile([P, D], BF16, tag="att")
                for g in range(NUM_HEADS):
                    gs = slice(g * HEAD_DIM, (g + 1) * HEAD_DIM)
                    for h in range(NUM_HEADS):
                        hs = slice(h * HEAD_DIM, (h + 1) * HEAD_DIM)
                        if g == 0:
                            nc.vector.tensor_scalar_mul(
                                att[:, hs], v_sb[:, gs],
                                w_sm[:, 4 * h:4 * h + 1],
                            )
                        else:
                            nc.vector.scalar_tensor_tensor(
                                out=att[:, hs],
                                in0=v_sb[:, gs],
                                scalar=w_sm[:, 4 * h + g:4 * h + g + 1],
                                in1=att[:, hs],
                                op0=ALU.mult,
                                op1=ALU.add,
                            )
                atts[t] = att

            def out_phase(t):
                att = atts[t]
                # ---- transpose attended (bf16, all 8 chunks in one bank) ----
                tr_ps = psum.tile([P, KC, P], BF16, tag="tr", bufs=1)
                for k in range(KC):
                    nc.tensor.transpose(
                        tr_ps[:, k], att[:, k * P:(k + 1) * P], ident,
                    )
                attT = attT_pool.tile([P, KC, P], BF16, tag="attT")
                nc.scalar.copy(attT, tr_ps)
                attTs[t] = attT

                # ---- O projection (bf16), bias added by gpsimd ----
                o_ps0 = psum.tile([P, 512], F32, tag="o")
                o_ps1 = psum.tile([P, 512], F32, tag="o")
                for k in range(KC):
                    nc.tensor.matmul(o_ps0, attT[:, k], wo_sb[:, k, 0:512],
                                     start=(k == 0), stop=(k == KC - 1))
                    nc.tensor.matmul(o_ps1, attT[:, k], wo_sb[:, k, 512:1024],
                                     start=(k == 0), stop=(k == KC - 1))
                o_sb = o_pool.tile([P, D], F32, tag="o_sb")
                nc.scalar.copy(o_sb[:, 0:512], o_ps0)
                nc.scalar.copy(o_sb[:, 512:1024], o_ps1)
                nc.gpsimd.tensor_tensor(
                    out=o_sb, in0=o_sb, in1=brep_sb[:, 3], op=ALU.add)
                nc.sync.dma_start(out_d[t * P:(t + 1) * P, :], o_sb)

            for i in range(NT + 2):
                if i < NT:
                    proj_phase(i)
                if i >= 2:
                    out_phase(i - 2)

    nc.compile()
    return nc


def _prep_inputs(x, Wq, bq, Wk, bk, Wv, bv, Wo, bo):
    """Per-core input maps: xT tiles per batch element + replicated weights."""
    x = np.asarray(x, dtype=np.float32)
    f8 = ml_dtypes.float8_e4m3
    bf = ml_dtypes.bfloat16

    Wq = np.asarray(Wq, np.float32); bq = np.asarray(bq, np.float32)
    Wk = np.asarray(Wk, np.float32); bk = np.asarray(bk, np.float32)
    Wv = np.asarray(Wv, np.float32); bv = np.asarray(bv, np.float32)
    Wo = np.asarray(Wo, np.float32); bo = np.asarray(bo, np.float32)

    # fp8 weights: [D, D] -> [P, KC2, 2, D]
    def to8(W):
        return np.ascontiguousarray(
            (W * W_SCALE).reshape(KC2, 2, P, D).transpose(2, 0, 1, 3)
        ).astype(f8)

    wq8_h = to8(Wq)
    wk8_h = to8(Wk)

    wv_h = np.ascontiguousarray(
        Wv.reshape(KC, P, D).transpose(1, 0, 2)).astype(bf)
    wo_h = np.ascontiguousarray(
        Wo.reshape(KC, P, D).transpose(1, 0, 2)).astype(bf)

    # replicated bias rows: q/k pre-scaled to the fp8 psum scale (2^11)
    brep_h = np.stack([
        np.broadcast_to(bq * (X_SCALE * W_SCALE), (P, D)),
        np.broadcast_to(bk * (X_SCALE * W_SCALE), (P, D)),
        np.broadcast_to(bv, (P, D)),
        np.broadcast_to(bo, (P, D)),
    ], axis=1)
    brep_h = np.ascontiguousarray(brep_h).astype(bf)

    in_maps = []
    for b in range(B):
        xt = np.ascontiguousarray(
            x[b].T.reshape(KC, P, NT, P).transpose(2, 1, 0, 3))
        xtbf = xt.astype(bf)
        xt8 = (xt * X_SCALE).astype(f8).reshape(NT, P, KC2, 2, P)
        in_maps.append({
            "xt8": xt8, "xtbf": xtbf,
            "wq8": wq8_h, "wk8": wk8_h, "wv": wv_h, "wo": wo_h,
            "brep": brep_h,
        })
    return in_maps


def kernel(**inputs):
    global _CACHED_NC
    if _CACHED_NC is None:
        _CACHED_NC = build_nc()
    nc = _CACHED_NC

    in_maps = _prep_inputs(
        inputs["x"],
        inputs["Wq"], inputs["bq"],
        inputs["Wk"], inputs["bk"],
        inputs["Wv"], inputs["bv"],
        inputs["Wo"], inputs["bo"],
    )
    global LAST_RESULT
    res = run_bass_kernel_spmd(
        nc, in_maps, core_ids=list(range(B)),
        trace=TRACE, tmpdir=TRACE_DIR,
    )
    LAST_RESULT = res
    out = np.stack([r["out"] for r in res.results], axis=0)
    return out.astype(np.float32)


# revision 39
# speedup vs baseline: 1.3781x; 1.3781x over previous
"""Trainium2 Bass kernel for CryptoAttentionLayer.

Computation (per batch element b, per token t):
    Q = x @ Wq + bq ; K = x @ Wk + bk ; V = x @ Wv + bv    (4 heads x 256)
    S[h,g]   = Q_h . K_g / 16                               (per-token 4x4 scores)
    W        = softmax_g(S)
    att_h    = sum_g W[h,g] * V_g
    out      = att @ Wo + bo

Sharding: data-parallel over B=8 across 8 NeuronCores; weights replicated.

Design notes:
  - Q/K projections run in fp8e4 with perf_mode=DoubleRow (2 fp8/cell, K=256
    per pass).  x is pre-scaled by 2^4 and Wq/Wk by 2^7 so values sit in the
    e4m3 normal range; the descale plus the 1/sqrt(256) score scale ride the
    softmax exp's scale operand.  All four biases ride partition0-ones
    matmuls into their PSUM accumulations (q/k biases pre-scaled by 2^11).
  - V/O projections run in bf16.
  - Scores: one DVE tensor_tensor (Q broadcast over g x K broadcast over h),
    two bf16 tree-fold adds (tensor_reduce ADD only runs at 1x; TT adds hit
    the 2x bf16 mode), then a short tensor_reduce.  Softmax over 4 head
    pairs, no max-subtract (scores are O(1)).
  - Head mixing on DVE in bf16; PE transposes att for the O projection.
  - 2-deep software pipeline: transposes + O-projection of tile i-2 are
    issued after the projections of tile i, so the PE never waits on the
    vector engine's attention math.
  - Engine budget per 128-token tile: PE ~12.8us (at 100% stream efficiency,
    216ns per N=512 matmul), DVE ~11.3us, Scalar ~6.5us.  gpsimd is left
    idle on purpose: it shares its SBUF port with the DVE, and loading it
    measurably slows DVE ops (~2x on tensor_tensor).
"""

import math

import numpy as np
import ml_dtypes

import concourse.bass as bass
import concourse.tile as tile
import concourse.mybir as mybir
from concourse import bacc
from concourse.bass_utils import run_bass_kernel_spmd
from concourse.masks import make_identity

B, N, D = 8, 4096, 1024
NUM_HEADS, HEAD_DIM = 4, 256
P = 128
NT = N // P          # 32 token tiles per core
KC = D // P          # 8 contraction chunks of 128
KC2 = KC // 2        # 4 double-chunks for fp8 DoubleRow
F32 = mybir.dt.float32
BF16 = mybir.dt.bfloat16
FP8 = mybir.dt.float8e4
ALU = mybir.AluOpType
ACTF = mybir.ActivationFunctionType
DR = mybir.MatmulPerfMode.DoubleRow

X_SCALE = 16.0       # 2^4  : x pre-scale for fp8
W_SCALE = 128.0      # 2^7  : Wq/Wk pre-scale for fp8
DESCALE = 1.0 / (X_SCALE * X_SCALE * W_SCALE * W_SCALE * math.sqrt(HEAD_DIM))

_CACHED_NC = None

# test.py can set these to capture a perfetto trace + HW exec time; the
# grading harness never touches them.
TRACE = False
TRACE_DIR = None
LAST_RESULT = None


def build_nc():
    nc = bacc.Bacc(None, target_bir_lowering=False)

    xt8_d = nc.dram_tensor("xt8", [NT, P, KC2, 2, P], FP8, kind="ExternalInput")
    xtbf_d = nc.dram_tensor("xtbf", [NT, P, KC, P], BF16, kind="ExternalInput")
    wq8_d = nc.dram_tensor("wq8", [P, KC2, 2, D], FP8, kind="ExternalInput")
    wk8_d = nc.dram_tensor("wk8", [P, KC2, 2, D], FP8, kind="ExternalInput")
    wv_d = nc.dram_tensor("wv", [P, KC + 1, D], BF16, kind="ExternalInput")
    wo_d = nc.dram_tensor("wo", [P, KC + 1, D], BF16, kind="ExternalInput")
    bqk8_d = nc.dram_tensor("bqk8", [P, 2, D], FP8, kind="ExternalInput")
    ones_d = nc.dram_tensor("ones", [P, P], BF16, kind="ExternalInput")
    ones8_d = nc.dram_tensor("ones8", [P, P], FP8, kind="ExternalInput")
    out_d = nc.dram_tensor("out", [N, D], F32, kind="ExternalOutput")

    with tile.TileContext(nc) as tc:
        with (
            tc.tile_pool(name="consts", bufs=1) as consts,
            tc.tile_pool(name="xt8", bufs=2) as xt8_pool,
            tc.tile_pool(name="xtbf", bufs=2) as xtbf_pool,
            tc.tile_pool(name="qk", bufs=2) as qk_pool,
            tc.tile_pool(name="v", bufs=2) as v_pool,
            tc.tile_pool(name="att", bufs=3) as att_pool,
            tc.tile_pool(name="attT", bufs=2) as attT_pool,
            tc.tile_pool(name="o", bufs=2) as o_pool,
            tc.tile_pool(name="small", bufs=2) as small,
            tc.tile_pool(name="psum", bufs=2, space="PSUM") as psum,
        ):
            wq8_sb = consts.tile([P, KC2, 2, D], FP8)
            wk8_sb = consts.tile([P, KC2, 2, D], FP8)
            wv_sb = consts.tile([P, KC + 1, D], BF16)
            wo_sb = consts.tile([P, KC + 1, D], BF16)
            bqk8_sb = consts.tile([P, 2, D], FP8)
            ones_bf = consts.tile([P, P], BF16)
            ones8 = consts.tile([P, P], FP8)
            # critical-path weights on the sync DMA queue (chunked so the
            # first Q matmuls can start as soon as chunk 0 + xt8(0) land);
            # the rest on the scalar hwdge queue.
            for c in range(KC2):
                nc.sync.dma_start(wq8_sb[:, c], wq8_d[:, c])
            nc.sync.dma_start(bqk8_sb, bqk8_d[:])
            nc.sync.dma_start(ones8, ones8_d[:])
            nc.scalar.dma_start(wk8_sb, wk8_d[:])
            nc.scalar.dma_start(wv_sb, wv_d[:])
            nc.scalar.dma_start(ones_bf, ones_d[:])
            nc.scalar.dma_start(wo_sb, wo_d[:])

            ident = consts.tile([P, P], BF16)
            make_identity(nc, ident)

            atts = [None] * NT
            attTs = [None] * NT

            def proj_phase(t):
                xt8 = xt8_pool.tile([P, KC2, 2, P], FP8, tag="xt8")
                nc.sync.dma_start(xt8, xt8_d[t])
                xtbf = xtbf_pool.tile([P, KC, P], BF16, tag="xtbf")
                nc.sync.dma_start(xtbf, xtbf_d[t])

                # ---- Q/K projections: fp8 DoubleRow, K=256 per pass ----
                q_sb = qk_pool.tile([P, D], BF16, tag="q")
                k_sb = qk_pool.tile([P, D], BF16, tag="k")
                for qk, (w8, dst) in enumerate(
                        ((wq8_sb, q_sb), (wk8_sb, k_sb))):
                    ps0 = psum.tile([P, 512], F32, tag="qk", bufs=3)
                    ps1 = psum.tile([P, 512], F32, tag="qk", bufs=3)
                    for c in range(KC2):
                        nc.tensor.matmul(
                            ps0, xt8[:, c], w8[:, c, :, 0:512],
                            start=(c == 0), stop=False, perf_mode=DR,
                        )
                        nc.tensor.matmul(
                            ps1, xt8[:, c], w8[:, c, :, 512:1024],
                            start=(c == 0), stop=False, perf_mode=DR,
                        )
                    # bias (pre-scaled by 2^11, fp8) via partition0-ones
                    nc.tensor.matmul(ps0, ones8, bqk8_sb[:, qk, 0:512],
                                     start=False, stop=True)
                    nc.tensor.matmul(ps1, ones8, bqk8_sb[:, qk, 512:1024],
                                     start=False, stop=True)
                    nc.scalar.copy(dst[:, 0:512], ps0)
                    nc.scalar.copy(dst[:, 512:1024], ps1)

                # ---- V projection (bf16) + S_bias (N=16), shared ldweights ----
                v_ps0 = psum.tile([P, 512], F32, tag="b", bufs=2)
                v_ps1 = psum.tile([P, 512], F32, tag="b", bufs=2)
                for k in range(KC):
                    nc.tensor.matmul(v_ps0, xtbf[:, k], wv_sb[:, k, 0:512],
                                     start=(k == 0), stop=False)
                    nc.tensor.matmul(v_ps1, xtbf[:, k], wv_sb[:, k, 512:1024],
                                     start=(k == 0), stop=False)
                nc.tensor.matmul(v_ps0, ones_bf, wv_sb[:, KC, 0:512],
                                 start=False, stop=True)
                nc.tensor.matmul(v_ps1, ones_bf, wv_sb[:, KC, 512:1024],
                                 start=False, stop=True)
                v_sb = v_pool.tile([P, D], BF16, tag="v")
                nc.scalar.copy(v_sb[:, 0:512], v_ps0)
                nc.scalar.copy(v_sb[:, 512:1024], v_ps1)

                # ---- scores: prod = Q_h * K_g (bcast), tree-reduce over d ----
                # (tensor_reduce ADD runs at 1x; TT adds hit the 2x bf16 mode,
                # so fold 256 -> 128 -> 64 with TT before the final reduce)
                prod = small.tile([P, NUM_HEADS, NUM_HEADS, HEAD_DIM], BF16,
                                  tag="prod")
                q4 = q_sb.rearrange("p (h d) -> p h d", h=NUM_HEADS)
                k4 = k_sb.rearrange("p (g d) -> p g d", g=NUM_HEADS)
                nc.vector.tensor_tensor(
                    out=prod,
                    in0=q4[:, :, None, :].to_broadcast(
                        (P, NUM_HEADS, NUM_HEADS, HEAD_DIM)),
                    in1=k4[:, None, :, :].to_broadcast(
                        (P, NUM_HEADS, NUM_HEADS, HEAD_DIM)),
                    op=ALU.mult,
                )
                pr2 = prod.rearrange("p h g (i d) -> p (h g) i d", i=2)
                fold1 = small.tile([P, 16, 128], BF16, tag="fold1")
                nc.vector.tensor_tensor(
                    out=fold1, in0=pr2[:, :, 0], in1=pr2[:, :, 1], op=ALU.add)
                f2 = fold1.rearrange("p q (i d) -> p q i d", i=2)
                fold2 = small.tile([P, 16, 64], BF16, tag="fold2")
                nc.vector.tensor_tensor(
                    out=fold2, in0=f2[:, :, 0], in1=f2[:, :, 1], op=ALU.add)
                s_raw = small.tile([P, 16], F32, tag="sraw")
                nc.vector.tensor_reduce(
                    out=s_raw.rearrange("p (h g) -> p h g", g=NUM_HEADS),
                    in_=fold2.rearrange("p (h g) d -> p h g d", g=NUM_HEADS),
                    axis=mybir.AxisListType.X,
                    op=ALU.add,
                )
                # ---- softmax over g (scores are O(1); no max-subtract);
                # the fp8/score descale rides the activation's scale ----
                e_sb = small.tile([P, 16], F32, tag="e")
                nc.scalar.activation(e_sb, s_raw, ACTF.Exp, scale=DESCALE)
                sums = small.tile([P, NUM_HEADS], F32, tag="sums")
                nc.vector.tensor_reduce(
                    out=sums,
                    in_=e_sb.rearrange("p (h g) -> p h g", g=NUM_HEADS),
                    axis=mybir.AxisListType.X,
                    op=ALU.add,
                )
                rec = small.tile([P, NUM_HEADS], F32, tag="rec")
                nc.vector.reciprocal(rec, sums)
                w_sm = small.tile([P, 16], F32, tag="w")
                nc.vector.tensor_tensor(
                    out=w_sm.rearrange("p (h g) -> p h g", g=NUM_HEADS),
                    in0=e_sb.rearrange("p (h g) -> p h g", g=NUM_HEADS),
                    in1=rec[:, :, None].to_broadcast(
                        (P, NUM_HEADS, NUM_HEADS)),
                    op=ALU.mult,
                )

                # ---- head mixing: att_h = sum_g w[h,g] * V_g (bf16) ----
                # g-outer so consecutive DVE ops are independent (no
                # serialization on the per-head accumulation chain); head 3
                # runs on the otherwise-idle gpsimd engine.
                att = att_pool.tile([P, D], BF16, tag="att")
                for g in range(NUM_HEADS):
                    gs = slice(g * HEAD_DIM, (g + 1) * HEAD_DIM)
                    for h in range(NUM_HEADS):
                        hs = slice(h * HEAD_DIM, (h + 1) * HEAD_DIM)
                        if g == 0:
                            nc.vector.tensor_scalar_mul(
                                att[:, hs], v_sb[:, gs],
                                w_sm[:, 4 * h:4 * h + 1],
                            )
                        else:
                            nc.vector.scalar_tensor_tensor(
                                out=att[:, hs],
                                in0=v_sb[:, gs],
                                scalar=w_sm[:, 4 * h + g:4 * h + g + 1],
                                in1=att[:, hs],
                                op0=ALU.mult,
                                op1=ALU.add,
                            )
                atts[t] = att

            def out_phase(t):
                att = atts[t]
                # ---- transpose attended (bf16, all 8 chunks in one bank) ----
                tr_ps = psum.tile([P, KC, P], BF16, tag="tr", bufs=1)
                for k in range(KC):
                    nc.tensor.transpose(
                        tr_ps[:, k], att[:, k * P:(k + 1) * P], ident,
                    )
                attT = attT_pool.tile([P, KC, P], BF16, tag="attT")
                nc.scalar.copy(attT, tr_ps)
                attTs[t] = attT

                # ---- O projection (bf16), bias via ones-row matmul ----
                o_ps0 = psum.tile([P, 512], F32, tag="o")
                o_ps1 = psum.tile([P, 512], F32, tag="o")
                for k in range(KC):
                    nc.tensor.matmul(o_ps0, attT[:, k], wo_sb[:, k, 0:512],
                                     start=(k == 0), stop=False)
                    nc.tensor.matmul(o_ps1, attT[:, k], wo_sb[:, k, 512:1024],
                                     start=(k == 0), stop=False)
                nc.tensor.matmul(o_ps0, ones_bf, wo_sb[:, KC, 0:512],
                                 start=False, stop=True)
                nc.tensor.matmul(o_ps1, ones_bf, wo_sb[:, KC, 512:1024],
                                 start=False, stop=True)
                o_sb = o_pool.tile([P, D], F32, tag="o_sb")
                nc.scalar.copy(o_sb[:, 0:512], o_ps0)
                nc.scalar.copy(o_sb[:, 512:1024], o_ps1)
                nc.sync.dma_start(out_d[t * P:(t + 1) * P, :], o_sb)

            for i in range(NT + 2):
                if i < NT:
                    proj_phase(i)
                if i >= 2:
                    out_phase(i - 2)

    nc.compile()
    return nc


def _prep_inputs(x, Wq, bq, Wk, bk, Wv, bv, Wo, bo):
    """Per-core input maps: xT tiles per batch element + replicated weights."""
    x = np.asarray(x, dtype=np.float32)
    f8 = ml_dtypes.float8_e4m3
    bf = ml_dtypes.bfloat16

    Wq = np.asarray(Wq, np.float32); bq = np.asarray(bq, np.float32)
    Wk = np.asarray(Wk, np.float32); bk = np.asarray(bk, np.float32)
    Wv = np.asarray(Wv, np.float32); bv = np.asarray(bv, np.float32)
    Wo = np.asarray(Wo, np.float32); bo = np.asarray(bo, np.float32)

    # fp8 weights: [D, D] -> [P, KC2, 2, D]
    def to8(W):
        return np.ascontiguousarray(
            (W * W_SCALE).reshape(KC2, 2, P, D).transpose(2, 0, 1, 3)
        ).astype(f8)

    wq8_h = to8(Wq)
    wk8_h = to8(Wk)

    wv_h = np.ascontiguousarray(
        np.concatenate([Wv, bv[None, :], np.zeros((P - 1, D), np.float32)],
                       axis=0).reshape(KC + 1, P, D).transpose(1, 0, 2)
    ).astype(bf)
    wo_h = np.ascontiguousarray(
        np.concatenate([Wo, bo[None, :], np.zeros((P - 1, D), np.float32)],
                       axis=0).reshape(KC + 1, P, D).transpose(1, 0, 2)
    ).astype(bf)

    # Q/K biases pre-scaled to match the fp8 psum scale (2^11), fp8
    bqk8_h = np.zeros((P, 2, D), np.float32)
    bqk8_h[0, 0] = bq * (X_SCALE * W_SCALE)
    bqk8_h[0, 1] = bk * (X_SCALE * W_SCALE)
    bqk8_h = bqk8_h.astype(f8)

    ones_h = np.zeros((P, P), np.float32)
    ones_h[0, :] = 1.0
    ones8_h = ones_h.astype(f8)
    ones_h = ones_h.astype(bf)

    in_maps = []
    for b in range(B):
        xt = np.ascontiguousarray(
            x[b].T.reshape(KC, P, NT, P).transpose(2, 1, 0, 3))
        xtbf = xt.astype(bf)
        xt8 = (xt * X_SCALE).astype(f8).reshape(NT, P, KC2, 2, P)
        in_maps.append({
            "xt8": xt8, "xtbf": xtbf,
            "wq8": wq8_h, "wk8": wk8_h, "wv": wv_h, "wo": wo_h,
            "bqk8": bqk8_h, "ones": ones_h, "ones8": ones8_h,
        })
    return in_maps


def kernel(**inputs):
    global _CACHED_NC
    if _CACHED_NC is None:
        _CACHED_NC = build_nc()
    nc = _CACHED_NC

    in_maps = _prep_inputs(
        inputs["x"],
        inputs["Wq"], inputs["bq"],
        inputs["Wk"], inputs["bk"],
        inputs["Wv"], inputs["bv"],
        inputs["Wo"], inputs["bo"],
    )
    global LAST_RESULT
    res = run_bass_kernel_spmd(
        nc, in_maps, core_ids=list(range(B)),
        trace=TRACE, tmpdir=TRACE_DIR,
    )
    LAST_RESULT = res
    out = np.stack([r["out"] for r in res.results], axis=0)
    return out.astype(np.float32)


# revision 41
# speedup vs baseline: 1.3812x; 1.0023x over previous
"""Trainium2 Bass kernel for CryptoAttentionLayer.

Computation (per batch element b, per token t):
    Q = x @ Wq + bq ; K = x @ Wk + bk ; V = x @ Wv + bv    (4 heads x 256)
    S[h,g]   = Q_h . K_g / 16                               (per-token 4x4 scores)
    W        = softmax_g(S)
    att_h    = sum_g W[h,g] * V_g
    out      = att @ Wo + bo

Sharding: data-parallel over B=8 across 8 NeuronCores; weights replicated.

Design notes:
  - Q/K projections run in fp8e4 with perf_mode=DoubleRow (2 fp8/cell, K=256
    per pass).  x is pre-scaled by 2^4 and Wq/Wk by 2^7 so values sit in the
    e4m3 normal range; the descale plus the 1/sqrt(256) score scale ride the
    softmax exp's scale operand.  All four biases ride partition0-ones
    matmuls into their PSUM accumulations (q/k biases pre-scaled by 2^11).
  - V/O projections run in bf16.
  - Scores: one DVE tensor_tensor (Q broadcast over g x K broadcast over h),
    two bf16 tree-fold adds (tensor_reduce ADD only runs at 1x; TT adds hit
    the 2x bf16 mode), then a short tensor_reduce.  Softmax over 4 head
    pairs, no max-subtract (scores are O(1)).
  - Head mixing on DVE in bf16; PE transposes att for the O projection.
  - 2-deep software pipeline: transposes + O-projection of tile i-2 are
    issued after the projections of tile i, so the PE never waits on the
    vector engine's attention math.
  - Engine budget per 128-token tile: PE ~12.8us (at 100% stream efficiency,
    216ns per N=512 matmul), DVE ~11.3us, Scalar ~6.5us.  gpsimd is left
    idle on purpose: it shares its SBUF port with the DVE, and loading it
    measurably slows DVE ops (~2x on tensor_tensor).
"""

import math

import numpy as np
import ml_dtypes

import concourse.bass as bass
import concourse.tile as tile
import concourse.mybir as mybir
from concourse import bacc
from concourse.bass_utils import run_bass_kernel_spmd
from concourse.masks import make_identity

B, N, D = 8, 4096, 1024
NUM_HEADS, HEAD_DIM = 4, 256
P = 128
NT = N // P          # 32 token tiles per core
KC = D // P          # 8 contraction chunks of 128
KC2 = KC // 2        # 4 double-chunks for fp8 DoubleRow
F32 = mybir.dt.float32
BF16 = mybir.dt.bfloat16
FP8 = mybir.dt.float8e4
ALU = mybir.AluOpType
ACTF = mybir.ActivationFunctionType
DR = mybir.MatmulPerfMode.DoubleRow

X_SCALE = 16.0       # 2^4  : x pre-scale for fp8
W_SCALE = 128.0      # 2^7  : Wq/Wk pre-scale for fp8
DESCALE = 1.0 / (X_SCALE * X_SCALE * W_SCALE * W_SCALE * math.sqrt(HEAD_DIM))

_CACHED_NC = None

# test.py can set these to capture a perfetto trace + HW exec time; the
# grading harness never touches them.
TRACE = False
TRACE_DIR = None
LAST_RESULT = None


def build_nc():
    nc = bacc.Bacc(None, target_bir_lowering=False)

    xt8_d = nc.dram_tensor("xt8", [NT, P, KC2, 2, P], FP8, kind="ExternalInput")
    xtbf_d = nc.dram_tensor("xtbf", [NT, P, KC, P], BF16, kind="ExternalInput")
    wq8_d = nc.dram_tensor("wq8", [P, KC2, 2, D], FP8, kind="ExternalInput")
    wk8_d = nc.dram_tensor("wk8", [P, KC2, 2, D], FP8, kind="ExternalInput")
    wv_d = nc.dram_tensor("wv", [P, KC + 1, D], BF16, kind="ExternalInput")
    wo_d = nc.dram_tensor("wo", [P, KC + 1, D], BF16, kind="ExternalInput")
    bqk8_d = nc.dram_tensor("bqk8", [P, 2, D], FP8, kind="ExternalInput")
    ones_d = nc.dram_tensor("ones", [P, P], BF16, kind="ExternalInput")
    ones8_d = nc.dram_tensor("ones8", [P, P], FP8, kind="ExternalInput")
    out_d = nc.dram_tensor("out", [N, D], F32, kind="ExternalOutput")

    with tile.TileContext(nc) as tc:
        with (
            tc.tile_pool(name="consts", bufs=1) as consts,
            tc.tile_pool(name="xt8", bufs=2) as xt8_pool,
            tc.tile_pool(name="xtbf", bufs=2) as xtbf_pool,
            tc.tile_pool(name="qk", bufs=2) as qk_pool,
            tc.tile_pool(name="v", bufs=2) as v_pool,
            tc.tile_pool(name="att", bufs=3) as att_pool,
            tc.tile_pool(name="attT", bufs=2) as attT_pool,
            tc.tile_pool(name="o", bufs=2) as o_pool,
            tc.tile_pool(name="small", bufs=2) as small,
            tc.tile_pool(name="psum", bufs=2, space="PSUM") as psum,
        ):
            wq8_sb = consts.tile([P, KC2, 2, D], FP8)
            wk8_sb = consts.tile([P, KC2, 2, D], FP8)
            wv_sb = consts.tile([P, KC + 1, D], BF16)
            wo_sb = consts.tile([P, KC + 1, D], BF16)
            bqk8_sb = consts.tile([P, 2, D], FP8)
            ones_bf = consts.tile([P, P], BF16)
            ones8 = consts.tile([P, P], FP8)
            def load_x(t):
                xt8 = xt8_pool.tile([P, KC2, 2, P], FP8, tag="xt8",
                                    name=f"xt8_{t}")
                nc.sync.dma_start(xt8, xt8_d[t])
                xtbf = xtbf_pool.tile([P, KC, P], BF16, tag="xtbf",
                                      name=f"xtbf_{t}")
                nc.sync.dma_start(xtbf, xtbf_d[t])
                return xt8, xtbf

            # critical path to the first matmul: wq8 chunk 0 + xt8(0).
            # Everything else follows on the sync queue or rides the scalar
            # hwdge queue.
            nc.sync.dma_start(wq8_sb[:, 0], wq8_d[:, 0])
            x0 = load_x(0)
            for c in range(1, KC2):
                nc.sync.dma_start(wq8_sb[:, c], wq8_d[:, c])
            nc.sync.dma_start(bqk8_sb, bqk8_d[:])
            nc.sync.dma_start(ones8, ones8_d[:])
            nc.scalar.dma_start(wk8_sb, wk8_d[:])
            nc.scalar.dma_start(wv_sb, wv_d[:])
            nc.scalar.dma_start(ones_bf, ones_d[:])
            nc.scalar.dma_start(wo_sb, wo_d[:])

            ident = consts.tile([P, P], BF16)
            make_identity(nc, ident)

            atts = [None] * NT
            attTs = [None] * NT

            def proj_phase(t, x_tiles=None):
                xt8, xtbf = x_tiles if x_tiles is not None else load_x(t)

                # ---- Q/K projections: fp8 DoubleRow, K=256 per pass ----
                q_sb = qk_pool.tile([P, D], BF16, tag="q")
                k_sb = qk_pool.tile([P, D], BF16, tag="k")
                for qk, (w8, dst) in enumerate(
                        ((wq8_sb, q_sb), (wk8_sb, k_sb))):
                    ps0 = psum.tile([P, 512], F32, tag="qk", bufs=3)
                    ps1 = psum.tile([P, 512], F32, tag="qk", bufs=3)
                    for c in range(KC2):
                        nc.tensor.matmul(
                            ps0, xt8[:, c], w8[:, c, :, 0:512],
                            start=(c == 0), stop=False, perf_mode=DR,
                        )
                        nc.tensor.matmul(
                            ps1, xt8[:, c], w8[:, c, :, 512:1024],
                            start=(c == 0), stop=False, perf_mode=DR,
                        )
                    # bias (pre-scaled by 2^11, fp8) via partition0-ones
                    nc.tensor.matmul(ps0, ones8, bqk8_sb[:, qk, 0:512],
                                     start=False, stop=True)
                    nc.tensor.matmul(ps1, ones8, bqk8_sb[:, qk, 512:1024],
                                     start=False, stop=True)
                    nc.scalar.copy(dst[:, 0:512], ps0)
                    nc.scalar.copy(dst[:, 512:1024], ps1)

                # ---- V projection (bf16) + S_bias (N=16), shared ldweights ----
                v_ps0 = psum.tile([P, 512], F32, tag="b", bufs=2)
                v_ps1 = psum.tile([P, 512], F32, tag="b", bufs=2)
                for k in range(KC):
                    nc.tensor.matmul(v_ps0, xtbf[:, k], wv_sb[:, k, 0:512],
                                     start=(k == 0), stop=False)
                    nc.tensor.matmul(v_ps1, xtbf[:, k], wv_sb[:, k, 512:1024],
                                     start=(k == 0), stop=False)
                nc.tensor.matmul(v_ps0, ones_bf, wv_sb[:, KC, 0:512],
                                 start=False, stop=True)
                nc.tensor.matmul(v_ps1, ones_bf, wv_sb[:, KC, 512:1024],
                                 start=False, stop=True)
                v_sb = v_pool.tile([P, D], BF16, tag="v")
                nc.scalar.copy(v_sb[:, 0:512], v_ps0)
                nc.scalar.copy(v_sb[:, 512:1024], v_ps1)

                # ---- scores: prod = Q_h * K_g (bcast), tree-reduce over d ----
                # (tensor_reduce ADD runs at 1x; TT adds hit the 2x bf16 mode,
                # so fold 256 -> 128 -> 64 with TT before the final reduce)
                prod = small.tile([P, NUM_HEADS, NUM_HEADS, HEAD_DIM], BF16,
                                  tag="prod")
                q4 = q_sb.rearrange("p (h d) -> p h d", h=NUM_HEADS)
                k4 = k_sb.rearrange("p (g d) -> p g d", g=NUM_HEADS)
                nc.vector.tensor_tensor(
                    out=prod,
                    in0=q4[:, :, None, :].to_broadcast(
                        (P, NUM_HEADS, NUM_HEADS, HEAD_DIM)),
                    in1=k4[:, None, :, :].to_broadcast(
                        (P, NUM_HEADS, NUM_HEADS, HEAD_DIM)),
                    op=ALU.mult,
                )
                pr2 = prod.rearrange("p h g (i d) -> p (h g) i d", i=2)
                fold1 = small.tile([P, 16, 128], BF16, tag="fold1")
                nc.vector.tensor_tensor(
                    out=fold1, in0=pr2[:, :, 0], in1=pr2[:, :, 1], op=ALU.add)
                f2 = fold1.rearrange("p q (i d) -> p q i d", i=2)
                fold2 = small.tile([P, 16, 64], BF16, tag="fold2")
                nc.vector.tensor_tensor(
                    out=fold2, in0=f2[:, :, 0], in1=f2[:, :, 1], op=ALU.add)
                s_raw = small.tile([P, 16], F32, tag="sraw")
                nc.vector.tensor_reduce(
                    out=s_raw.rearrange("p (h g) -> p h g", g=NUM_HEADS),
                    in_=fold2.rearrange("p (h g) d -> p h g d", g=NUM_HEADS),
                    axis=mybir.AxisListType.X,
                    op=ALU.add,
                )
                # ---- softmax over g (scores are O(1); no max-subtract);
                # the fp8/score descale rides the activation's scale ----
                e_sb = small.tile([P, 16], F32, tag="e")
                nc.scalar.activation(e_sb, s_raw, ACTF.Exp, scale=DESCALE)
                sums = small.tile([P, NUM_HEADS], F32, tag="sums")
                nc.vector.tensor_reduce(
                    out=sums,
                    in_=e_sb.rearrange("p (h g) -> p h g", g=NUM_HEADS),
                    axis=mybir.AxisListType.X,
                    op=ALU.add,
                )
                rec = small.tile([P, NUM_HEADS], F32, tag="rec")
                nc.vector.reciprocal(rec, sums)
                w_sm = small.tile([P, 16], F32, tag="w")
                nc.vector.tensor_tensor(
                    out=w_sm.rearrange("p (h g) -> p h g", g=NUM_HEADS),
                    in0=e_sb.rearrange("p (h g) -> p h g", g=NUM_HEADS),
                    in1=rec[:, :, None].to_broadcast(
                        (P, NUM_HEADS, NUM_HEADS)),
                    op=ALU.mult,
                )

                # ---- head mixing: att_h = sum_g w[h,g] * V_g (bf16) ----
                # g-outer so consecutive DVE ops are independent (no
                # serialization on the per-head accumulation chain); head 3
                # runs on the otherwise-idle gpsimd engine.
                att = att_pool.tile([P, D], BF16, tag="att")
                for g in range(NUM_HEADS):
                    gs = slice(g * HEAD_DIM, (g + 1) * HEAD_DIM)
                    for h in range(NUM_HEADS):
                        hs = slice(h * HEAD_DIM, (h + 1) * HEAD_DIM)
                        if g == 0:
                            nc.vector.tensor_scalar_mul(
                                att[:, hs], v_sb[:, gs],
                                w_sm[:, 4 * h:4 * h + 1],
                            )
                        else:
                            nc.vector.scalar_tensor_tensor(
                                out=att[:, hs],
                                in0=v_sb[:, gs],
                                scalar=w_sm[:, 4 * h + g:4 * h + g + 1],
                                in1=att[:, hs],
                                op0=ALU.mult,
                                op1=ALU.add,
                            )
                atts[t] = att

            def out_phase(t):
                att = atts[t]
                # ---- transpose attended (bf16, all 8 chunks in one bank) ----
                tr_ps = psum.tile([P, KC, P], BF16, tag="tr", bufs=1)
                for k in range(KC):
                    nc.tensor.transpose(
                        tr_ps[:, k], att[:, k * P:(k + 1) * P], ident,
                    )
                attT = attT_pool.tile([P, KC, P], BF16, tag="attT")
                nc.scalar.copy(attT, tr_ps)
                attTs[t] = attT

                # ---- O projection (bf16), bias via ones-row matmul ----
                o_ps0 = psum.tile([P, 512], F32, tag="o")
                o_ps1 = psum.tile([P, 512], F32, tag="o")
                for k in range(KC):
                    nc.tensor.matmul(o_ps0, attT[:, k], wo_sb[:, k, 0:512],
                                     start=(k == 0), stop=False)
                    nc.tensor.matmul(o_ps1, attT[:, k], wo_sb[:, k, 512:1024],
                                     start=(k == 0), stop=False)
                nc.tensor.matmul(o_ps0, ones_bf, wo_sb[:, KC, 0:512],
                                 start=False, stop=True)
                nc.tensor.matmul(o_ps1, ones_bf, wo_sb[:, KC, 512:1024],
                                 start=False, stop=True)
                o_sb = o_pool.tile([P, D], F32, tag="o_sb")
                nc.scalar.copy(o_sb[:, 0:512], o_ps0)
                nc.scalar.copy(o_sb[:, 512:1024], o_ps1)
                nc.sync.dma_start(out_d[t * P:(t + 1) * P, :], o_sb)

            for i in range(NT + 2):
                if i < NT:
                    proj_phase(i, x0 if i == 0 else None)
                if i >= 2:
                    out_phase(i - 2)

    nc.compile()
    return nc


def _prep_inputs(x, Wq, bq, Wk, bk, Wv, bv, Wo, bo):
    """Per-core input maps: xT tiles per batch element + replicated weights."""
    x = np.asarray(x, dtype=np.float32)
    f8 = ml_dtypes.float8_e4m3
    bf = ml_dtypes.bfloat16

    Wq = np.asarray(Wq, np.float32); bq = np.asarray(bq, np.float32)
    Wk = np.asarray(Wk, np.float32); bk = np.asarray(bk, np.float32)
    Wv = np.asarray(Wv, np.float32); bv = np.asarray(bv, np.float32)
    Wo = np.asarray(Wo, np.float32); bo = np.asarray(bo, np.float32)

    # fp8 weights: [D, D] -> [P, KC2, 2, D]
    def to8(W):
        return np.ascontiguousarray(
            (W * W_SCALE).reshape(KC2, 2, P, D).transpose(2, 0, 1, 3)
        ).astype(f8)

    wq8_h = to8(Wq)
    wk8_h = to8(Wk)

    wv_h = np.ascontiguousarray(
        np.concatenate([Wv, bv[None, :], np.zeros((P - 1, D), np.float32)],
                       axis=0).reshape(KC + 1, P, D).transpose(1, 0, 2)
    ).astype(bf)
    wo_h = np.ascontiguousarray(
        np.concatenate([Wo, bo[None, :], np.zeros((P - 1, D), np.float32)],
                       axis=0).reshape(KC + 1, P, D).transpose(1, 0, 2)
    ).astype(bf)

    # Q/K biases pre-scaled to match the fp8 psum scale (2^11), fp8
    bqk8_h = np.zeros((P, 2, D), np.float32)
    bqk8_h[0, 0] = bq * (X_SCALE * W_SCALE)
    bqk8_h[0, 1] = bk * (X_SCALE * W_SCALE)
    bqk8_h = bqk8_h.astype(f8)

    ones_h = np.zeros((P, P), np.float32)
    ones_h[0, :] = 1.0
    ones8_h = ones_h.astype(f8)
    ones_h = ones_h.astype(bf)

    in_maps = []
    for b in range(B):
        xt = np.ascontiguousarray(
            x[b].T.reshape(KC, P, NT, P).transpose(2, 1, 0, 3))
        xtbf = xt.astype(bf)
        xt8 = (xt * X_SCALE).astype(f8).reshape(NT, P, KC2, 2, P)
        in_maps.append({
            "xt8": xt8, "xtbf": xtbf,
            "wq8": wq8_h, "wk8": wk8_h, "wv": wv_h, "wo": wo_h,
            "bqk8": bqk8_h, "ones": ones_h, "ones8": ones8_h,
        })
    return in_maps


def kernel(**inputs):
    global _CACHED_NC
    if _CACHED_NC is None:
        _CACHED_NC = build_nc()
    nc = _CACHED_NC

    in_maps = _prep_inputs(
        inputs["x"],
        inputs["Wq"], inputs["bq"],
        inputs["Wk"], inputs["bk"],
        inputs["Wv"], inputs["bv"],
        inputs["Wo"], inputs["bo"],
    )
    global LAST_RESULT
    res = run_bass_kernel_spmd(
        nc, in_maps, core_ids=list(range(B)),
        trace=TRACE, tmpdir=TRACE_DIR,
    )
    LAST_RESULT = res
    out = np.stack([r["out"] for r in res.results], axis=0)
    return out.astype(np.float32)


# revision 45
# speedup vs baseline: 1.3894x; 1.0059x over previous
"""Trainium2 Bass kernel for CryptoAttentionLayer.

Computation (per batch element b, per token t):
    Q = x @ Wq + bq ; K = x @ Wk + bk ; V = x @ Wv + bv    (4 heads x 256)
    S[h,g]   = Q_h . K_g / 16                               (per-token 4x4 scores)
    W        = softmax_g(S)
    att_h    = sum_g W[h,g] * V_g
    out      = att @ Wo + bo

Sharding: data-parallel over B=8 across 8 NeuronCores; weights replicated.

Design notes:
  - Q/K projections run in fp8e4 with perf_mode=DoubleRow (2 fp8/cell, K=256
    per pass).  x is pre-scaled by 2^4 and Wq/Wk by 2^7 so values sit in the
    e4m3 normal range; the descale plus the 1/sqrt(256) score scale ride the
    softmax exp's scale operand.  All four biases ride partition0-ones
    matmuls into their PSUM accumulations (q/k biases pre-scaled by 2^11).
  - V/O projections run in bf16.
  - Scores: one DVE tensor_tensor (Q broadcast over g x K broadcast over h),
    two bf16 tree-fold adds (tensor_reduce ADD only runs at 1x; TT adds hit
    the 2x bf16 mode), then a short tensor_reduce.  Softmax over 4 head
    pairs, no max-subtract (scores are O(1)).
  - Head mixing on DVE in bf16; PE transposes att for the O projection.
  - 2-deep software pipeline: transposes + O-projection of tile i-2 are
    issued after the projections of tile i, so the PE never waits on the
    vector engine's attention math.
  - Engine budget per 128-token tile: PE ~12.8us (at 100% stream efficiency,
    216ns per N=512 matmul), DVE ~11.3us, Scalar ~6.5us.  gpsimd is left
    idle on purpose: it shares its SBUF port with the DVE, and loading it
    measurably slows DVE ops (~2x on tensor_tensor).
"""

import math

import numpy as np
import ml_dtypes

import concourse.bass as bass
import concourse.tile as tile
import concourse.mybir as mybir
from concourse import bacc
from concourse.bass_utils import run_bass_kernel_spmd
from concourse.masks import make_identity

B, N, D = 8, 4096, 1024
NUM_HEADS, HEAD_DIM = 4, 256
P = 128
NT = N // P          # 32 token tiles per core
KC = D // P          # 8 contraction chunks of 128
KC2 = KC // 2        # 4 double-chunks for fp8 DoubleRow
F32 = mybir.dt.float32
BF16 = mybir.dt.bfloat16
FP8 = mybir.dt.float8e4
ALU = mybir.AluOpType
ACTF = mybir.ActivationFunctionType
DR = mybir.MatmulPerfMode.DoubleRow

X_SCALE = 16.0       # 2^4  : x pre-scale for fp8
W_SCALE = 128.0      # 2^7  : Wq/Wk pre-scale for fp8
DESCALE = 1.0 / (X_SCALE * X_SCALE * W_SCALE * W_SCALE * math.sqrt(HEAD_DIM))

_CACHED_NC = None

# test.py can set these to capture a perfetto trace + HW exec time; the
# grading harness never touches them.
TRACE = False
TRACE_DIR = None
LAST_RESULT = None


def build_nc():
    nc = bacc.Bacc(None, target_bir_lowering=False)

    xt8_d = nc.dram_tensor("xt8", [NT, P, KC2, 2, P], FP8, kind="ExternalInput")
    xtbf_d = nc.dram_tensor("xtbf", [NT, P, KC, P], BF16, kind="ExternalInput")
    # fp8 weights with the contraction pair innermost (adjacent bytes), so
    # the DoubleRow moving-operand stream can fetch both values per lane in
    # one 16-bit read
    wq8_d = nc.dram_tensor("wq8", [P, KC2, D, 2], FP8, kind="ExternalInput")
    wk8_d = nc.dram_tensor("wk8", [P, KC2, D, 2], FP8, kind="ExternalInput")
    wv_d = nc.dram_tensor("wv", [P, KC + 1, D], BF16, kind="ExternalInput")
    wo_d = nc.dram_tensor("wo", [P, KC + 1, D], BF16, kind="ExternalInput")
    bqk8_d = nc.dram_tensor("bqk8", [P, 2, D], FP8, kind="ExternalInput")
    ones_d = nc.dram_tensor("ones", [P, P], BF16, kind="ExternalInput")
    ones8_d = nc.dram_tensor("ones8", [P, P], FP8, kind="ExternalInput")
    out_d = nc.dram_tensor("out", [N, D], F32, kind="ExternalOutput")

    with tile.TileContext(nc) as tc:
        with (
            tc.tile_pool(name="consts", bufs=1) as consts,
            tc.tile_pool(name="xt8", bufs=2) as xt8_pool,
            tc.tile_pool(name="xtbf", bufs=2) as xtbf_pool,
            tc.tile_pool(name="qk", bufs=2) as qk_pool,
            tc.tile_pool(name="v", bufs=2) as v_pool,
            tc.tile_pool(name="att", bufs=3) as att_pool,
            tc.tile_pool(name="attT", bufs=2) as attT_pool,
            tc.tile_pool(name="o", bufs=2) as o_pool,
            tc.tile_pool(name="small", bufs=2) as small,
            tc.tile_pool(name="psum", bufs=2, space="PSUM") as psum,
        ):
            wq8_sb = consts.tile([P, KC2, D, 2], FP8)
            wk8_sb = consts.tile([P, KC2, D, 2], FP8)
            wv_sb = consts.tile([P, KC + 1, D], BF16)
            wo_sb = consts.tile([P, KC + 1, D], BF16)
            bqk8_sb = consts.tile([P, 2, D], FP8)
            ones_bf = consts.tile([P, P], BF16)
            ones8 = consts.tile([P, P], FP8)
            def load_x(t):
                xt8 = xt8_pool.tile([P, KC2, 2, P], FP8, tag="xt8",
                                    name=f"xt8_{t}")
                nc.sync.dma_start(xt8, xt8_d[t])
                xtbf = xtbf_pool.tile([P, KC, P], BF16, tag="xtbf",
                                      name=f"xtbf_{t}")
                nc.sync.dma_start(xtbf, xtbf_d[t])
                return xt8, xtbf

            # critical path to the first matmul: wq8 chunk 0 + xt8(0).
            # Everything else follows on the sync queue or rides the scalar
            # hwdge queue.
            nc.sync.dma_start(wq8_sb[:, 0], wq8_d[:, 0])
            x0 = load_x(0)
            for c in range(1, KC2):
                nc.sync.dma_start(wq8_sb[:, c], wq8_d[:, c])
            nc.sync.dma_start(bqk8_sb, bqk8_d[:])
            nc.sync.dma_start(ones8, ones8_d[:])
            nc.scalar.dma_start(wk8_sb, wk8_d[:])
            nc.scalar.dma_start(wv_sb, wv_d[:])
            nc.scalar.dma_start(ones_bf, ones_d[:])
            nc.scalar.dma_start(wo_sb, wo_d[:])

            ident = consts.tile([P, P], BF16)
            make_identity(nc, ident)

            atts = [None] * NT
            attTs = [None] * NT

            def proj_phase(t, x_tiles=None):
                xt8, xtbf = x_tiles if x_tiles is not None else load_x(t)

                # ---- Q/K projections: fp8 DoubleRow, K=256 per pass ----
                q_sb = qk_pool.tile([P, D], BF16, tag="q")
                k_sb = qk_pool.tile([P, D], BF16, tag="k")
                for qk, (w8, dst) in enumerate(
                        ((wq8_sb, q_sb), (wk8_sb, k_sb))):
                    ps0 = psum.tile([P, 512], F32, tag="qk", bufs=3)
                    ps1 = psum.tile([P, 512], F32, tag="qk", bufs=3)
                    for c in range(KC2):
                        w8c = w8[:, c].rearrange("p n i -> p i n")
                        nc.tensor.matmul(
                            ps0, xt8[:, c], w8c[:, :, 0:512],
                            start=(c == 0), stop=False, perf_mode=DR,
                        )
                        nc.tensor.matmul(
                            ps1, xt8[:, c], w8c[:, :, 512:1024],
                            start=(c == 0), stop=False, perf_mode=DR,
                        )
                    # bias (pre-scaled by 2^11, fp8) via partition0-ones
                    nc.tensor.matmul(ps0, ones8, bqk8_sb[:, qk, 0:512],
                                     start=False, stop=True)
                    nc.tensor.matmul(ps1, ones8, bqk8_sb[:, qk, 512:1024],
                                     start=False, stop=True)
                    nc.scalar.copy(dst[:, 0:512], ps0)
                    nc.scalar.copy(dst[:, 512:1024], ps1)

                # ---- V projection (bf16) + S_bias (N=16), shared ldweights ----
                v_ps0 = psum.tile([P, 512], F32, tag="b", bufs=2)
                v_ps1 = psum.tile([P, 512], F32, tag="b", bufs=2)
                for k in range(KC):
                    nc.tensor.matmul(v_ps0, xtbf[:, k], wv_sb[:, k, 0:512],
                                     start=(k == 0), stop=False)
                    nc.tensor.matmul(v_ps1, xtbf[:, k], wv_sb[:, k, 512:1024],
                                     start=(k == 0), stop=False)
                nc.tensor.matmul(v_ps0, ones_bf, wv_sb[:, KC, 0:512],
                                 start=False, stop=True)
                nc.tensor.matmul(v_ps1, ones_bf, wv_sb[:, KC, 512:1024],
                                 start=False, stop=True)
                v_sb = v_pool.tile([P, D], BF16, tag="v")
                nc.scalar.copy(v_sb[:, 0:512], v_ps0)
                nc.scalar.copy(v_sb[:, 512:1024], v_ps1)

                # ---- scores: prod = Q_h * K_g (bcast), tree-reduce over d ----
                # (tensor_reduce ADD runs at 1x; TT adds hit the 2x bf16 mode,
                # so fold 256 -> 128 -> 64 with TT before the final reduce)
                prod = small.tile([P, NUM_HEADS, NUM_HEADS, HEAD_DIM], BF16,
                                  tag="prod")
                q4 = q_sb.rearrange("p (h d) -> p h d", h=NUM_HEADS)
                k4 = k_sb.rearrange("p (g d) -> p g d", g=NUM_HEADS)
                nc.vector.tensor_tensor(
                    out=prod,
                    in0=q4[:, :, None, :].to_broadcast(
                        (P, NUM_HEADS, NUM_HEADS, HEAD_DIM)),
                    in1=k4[:, None, :, :].to_broadcast(
                        (P, NUM_HEADS, NUM_HEADS, HEAD_DIM)),
                    op=ALU.mult,
                )
                pr2 = prod.rearrange("p h g (i d) -> p (h g) i d", i=2)
                fold1 = small.tile([P, 16, 128], BF16, tag="fold1")
                nc.vector.tensor_tensor(
                    out=fold1, in0=pr2[:, :, 0], in1=pr2[:, :, 1], op=ALU.add)
                f2 = fold1.rearrange("p q (i d) -> p q i d", i=2)
                fold2 = small.tile([P, 16, 64], BF16, tag="fold2")
                nc.vector.tensor_tensor(
                    out=fold2, in0=f2[:, :, 0], in1=f2[:, :, 1], op=ALU.add)
                s_raw = small.tile([P, 16], F32, tag="sraw")
                nc.vector.tensor_reduce(
                    out=s_raw.rearrange("p (h g) -> p h g", g=NUM_HEADS),
                    in_=fold2.rearrange("p (h g) d -> p h g d", g=NUM_HEADS),
                    axis=mybir.AxisListType.X,
                    op=ALU.add,
                )
                # ---- softmax over g (scores are O(1); no max-subtract);
                # the fp8/score descale rides the activation's scale ----
                e_sb = small.tile([P, 16], F32, tag="e")
                nc.scalar.activation(e_sb, s_raw, ACTF.Exp, scale=DESCALE)
                sums = small.tile([P, NUM_HEADS], F32, tag="sums")
                nc.vector.tensor_reduce(
                    out=sums,
                    in_=e_sb.rearrange("p (h g) -> p h g", g=NUM_HEADS),
                    axis=mybir.AxisListType.X,
                    op=ALU.add,
                )
                rec = small.tile([P, NUM_HEADS], F32, tag="rec")
                nc.vector.reciprocal(rec, sums)
                w_sm = small.tile([P, 16], F32, tag="w")
                nc.vector.tensor_tensor(
                    out=w_sm.rearrange("p (h g) -> p h g", g=NUM_HEADS),
                    in0=e_sb.rearrange("p (h g) -> p h g", g=NUM_HEADS),
                    in1=rec[:, :, None].to_broadcast(
                        (P, NUM_HEADS, NUM_HEADS)),
                    op=ALU.mult,
                )

                # ---- head mixing: att_h = sum_g w[h,g] * V_g (bf16) ----
                # g-outer so consecutive DVE ops are independent (no
                # serialization on the per-head accumulation chain); head 3
                # runs on the otherwise-idle gpsimd engine.
                att = att_pool.tile([P, D], BF16, tag="att")
                for g in range(NUM_HEADS):
                    gs = slice(g * HEAD_DIM, (g + 1) * HEAD_DIM)
                    for h in range(NUM_HEADS):
                        hs = slice(h * HEAD_DIM, (h + 1) * HEAD_DIM)
                        if g == 0:
                            nc.vector.tensor_scalar_mul(
                                att[:, hs], v_sb[:, gs],
                                w_sm[:, 4 * h:4 * h + 1],
                            )
                        else:
                            nc.vector.scalar_tensor_tensor(
                                out=att[:, hs],
                                in0=v_sb[:, gs],
                                scalar=w_sm[:, 4 * h + g:4 * h + g + 1],
                                in1=att[:, hs],
                                op0=ALU.mult,
                                op1=ALU.add,
                            )
                atts[t] = att

            def out_phase(t):
                att = atts[t]
                # ---- transpose attended (bf16, all 8 chunks in one bank) ----
                tr_ps = psum.tile([P, KC, P], BF16, tag="tr", bufs=1)
                for k in range(KC):
                    nc.tensor.transpose(
                        tr_ps[:, k], att[:, k * P:(k + 1) * P], ident,
                    )
                attT = attT_pool.tile([P, KC, P], BF16, tag="attT")
                nc.scalar.copy(attT, tr_ps)
                attTs[t] = attT

                # ---- O projection (bf16), bias via ones-row matmul ----
                o_ps0 = psum.tile([P, 512], F32, tag="o")
                o_ps1 = psum.tile([P, 512], F32, tag="o")
                for k in range(KC):
                    nc.tensor.matmul(o_ps0, attT[:, k], wo_sb[:, k, 0:512],
                                     start=(k == 0), stop=False)
                    nc.tensor.matmul(o_ps1, attT[:, k], wo_sb[:, k, 512:1024],
                                     start=(k == 0), stop=False)
                nc.tensor.matmul(o_ps0, ones_bf, wo_sb[:, KC, 0:512],
                                 start=False, stop=True)
                nc.tensor.matmul(o_ps1, ones_bf, wo_sb[:, KC, 512:1024],
                                 start=False, stop=True)
                o_sb = o_pool.tile([P, D], F32, tag="o_sb")
                nc.scalar.copy(o_sb[:, 0:512], o_ps0)
                nc.scalar.copy(o_sb[:, 512:1024], o_ps1)
                nc.sync.dma_start(out_d[t * P:(t + 1) * P, :], o_sb)

            for i in range(NT + 2):
                if i < NT:
                    proj_phase(i, x0 if i == 0 else None)
                if i >= 2:
                    out_phase(i - 2)

    nc.compile()
    return nc


def _prep_inputs(x, Wq, bq, Wk, bk, Wv, bv, Wo, bo):
    """Per-core input maps: xT tiles per batch element + replicated weights."""
    x = np.asarray(x, dtype=np.float32)
    f8 = ml_dtypes.float8_e4m3
    bf = ml_dtypes.bfloat16

    Wq = np.asarray(Wq, np.float32); bq = np.asarray(bq, np.float32)
    Wk = np.asarray(Wk, np.float32); bk = np.asarray(bk, np.float32)
    Wv = np.asarray(Wv, np.float32); bv = np.asarray(bv, np.float32)
    Wo = np.asarray(Wo, np.float32); bo = np.asarray(bo, np.float32)

    # fp8 weights: [D, D] -> [P, KC2, D, 2] (contraction pair innermost)
    def to8(W):
        return np.ascontiguousarray(
            (W * W_SCALE).reshape(KC2, 2, P, D).transpose(2, 0, 3, 1)
        ).astype(f8)

    wq8_h = to8(Wq)
    wk8_h = to8(Wk)

    wv_h = np.ascontiguousarray(
        np.concatenate([Wv, bv[None, :], np.zeros((P - 1, D), np.float32)],
                       axis=0).reshape(KC + 1, P, D).transpose(1, 0, 2)
    ).astype(bf)
    wo_h = np.ascontiguousarray(
        np.concatenate([Wo, bo[None, :], np.zeros((P - 1, D), np.float32)],
                       axis=0).reshape(KC + 1, P, D).transpose(1, 0, 2)
    ).astype(bf)

    # Q/K biases pre-scaled to match the fp8 psum scale (2^11), fp8
    bqk8_h = np.zeros((P, 2, D), np.float32)
    bqk8_h[0, 0] = bq * (X_SCALE * W_SCALE)
    bqk8_h[0, 1] = bk * (X_SCALE * W_SCALE)
    bqk8_h = bqk8_h.astype(f8)

    ones_h = np.zeros((P, P), np.float32)
    ones_h[0, :] = 1.0
    ones8_h = ones_h.astype(f8)
    ones_h = ones_h.astype(bf)

    in_maps = []
    for b in range(B):
        xt = np.ascontiguousarray(
            x[b].T.reshape(KC, P, NT, P).transpose(2, 1, 0, 3))
        xtbf = xt.astype(bf)
        xt8 = (xt * X_SCALE).astype(f8).reshape(NT, P, KC2, 2, P)
        in_maps.append({
            "xt8": xt8, "xtbf": xtbf,
            "wq8": wq8_h, "wk8": wk8_h, "wv": wv_h, "wo": wo_h,
            "bqk8": bqk8_h, "ones": ones_h, "ones8": ones8_h,
        })
    return in_maps


def kernel(**inputs):
    global _CACHED_NC
    if _CACHED_NC is None:
        _CACHED_NC = build_nc()
    nc = _CACHED_NC

    in_maps = _prep_inputs(
        inputs["x"],
        inputs["Wq"], inputs["bq"],
        inputs["Wk"], inputs["bk"],
        inputs["Wv"], inputs["bv"],
        inputs["Wo"], inputs["bo"],
    )
    global LAST_RESULT
    res = run_bass_kernel_spmd(
        nc, in_maps, core_ids=list(range(B)),
        trace=TRACE, tmpdir=TRACE_DIR,
    )
    LAST_RESULT = res
    out = np.stack([r["out"] for r in res.results], axis=0)
    return out.astype(np.float32)
